# revision 12
# baseline (speedup 1.0000x reference)
"""Trainium2 Bass kernel for nn_Adapter (moe_routing).

Reference computation (per router m in [0,12), batch b in [0,32)):
    e = expert_index[m, b]
    z = x[b] @ down_w[m, e] + down_b[m, e]     # [S, D]
    z = z * sigmoid(z)                          # SiLU
    u[m, b] = z @ up_w[m, e]                    # [S, C]

Strategy:
  - Data-parallel over batch B=32 across 8 cores (4 batches per core).
  - Expert routing (the gather over expert_index) is done on HOST: each
    core receives the already-gathered per-(m,b) weight tables, laid out
    exactly as the SBUF tiles want them, pre-cast to bf16.
  - Device, per (b, m-pair): routers are processed two at a time packed
    into the 128x128 PE array:
      * down-proj: z^T[D=64,S] for m0 -> PE cols 0-63, m1 -> cols 64-127
        (col tiling), accumulating over 8 K-chunks of C=1024.
      * SiLU+bias on the combined [128,S] PSUM tile in one ScalarE op,
        output bf16.
      * up-proj: m0 -> PE rows 0-63, m1 -> rows 64-127 (row tiling),
        K=D=64, interleaved so consecutive instructions overlap in the
        array.
    PSUM evicted to SBUF as bf16 by ScalarE/VectorE (alternating);
    two fully contiguous 512KiB DMAs out per (m,b) in [p, sc, c]
    layout which the host unpermutes. Input prefetch rides gpsimd's
    SWDGE queue so it never queues behind blocking output-DMA waits
    on the sync engine's HWDGE queue.
  - Output returned to host as bf16, host upcasts to f32 and stitches.
  - Perf: ~52 GFLOP as bf16 on 8 cores; per core 67.3MB of DRAM
    traffic is the roofline (~188us at the 358 GB/s per-core HBM spec);
    measured 181-224us run-to-run depending on chip-level HBM mode
    (337-383 GB/s effective), DMA busy 91-95% of the span.
"""

import os
import sys
import time

sys.path.insert(0, "/opt/trn_rl_repo")

import numpy as np
import ml_dtypes

M, N_EXP, C, D = 12, 8, 1024, 64
B, S = 32, 512
NCORES = 8
BL = B // NCORES          # batches per core = 4
KC = C // 128             # contraction chunks for down-proj = 8
SC = S // 128             # output row chunks for up-proj = 4
JP = M // 2               # router pairs per batch = 6

BF16 = ml_dtypes.bfloat16

# set by test.py to collect the profile
TRACE = bool(os.environ.get("KERNEL_TRACE"))
last_results = None

_nc_cache = {}


def _ensure_ntff_hook():
    """The agent image's `antenv` lacks `axon_hooks`, so the boot-time NTFF
    profile hook registration degrades silently and bass_utils' trace path
    crashes on import. Shim the module and install the ctypes hook."""
    import types

    if "antenv.axon_hooks" in sys.modules:
        return
    mod = types.ModuleType("antenv.axon_hooks")
    store = [None]
    mod.set_axon_ntff_profile_hook = lambda h: store.__setitem__(0, h)
    mod.get_axon_ntff_profile_hook = lambda: store[0]
    sys.modules["antenv.axon_hooks"] = mod
    try:
        import antenv

        antenv.axon_hooks = mod
    except ImportError:
        pass
    try:
        from trn_agent_boot.trn_boot import _ntff_profile_via_ctypes

        so_path = "/opt/axon/libaxon_pjrt.so"
        if os.path.exists(so_path):
            hook = _ntff_profile_via_ctypes(so_path)
            if hook is not None:
                mod.set_axon_ntff_profile_hook(hook)
    except Exception:
        pass


_ensure_ntff_hook()


def _build(variant=0):
    import concourse.mybir as mybir
    from concourse import bacc, tile

    bf16 = mybir.dt.bfloat16
    f32 = mybir.dt.float32
    AF = mybir.ActivationFunctionType

    nc = bacc.Bacc(
        "TRN2",
        target_bir_lowering=False,
        debug=False,
        num_devices=NCORES,
        num_swdge_queues=4,
    )
    xt_d = nc.declare_dram_parameter("xt", [BL, 128, KC, S], bf16, isOutput=False)
    wd_d = nc.declare_dram_parameter("wd", [BL, 128, JP, KC, 128], bf16, isOutput=False)
    wu_d = nc.declare_dram_parameter("wu", [BL, 128, JP, C], bf16, isOutput=False)
    bias_d = nc.declare_dram_parameter("bias", [BL, 128, JP], f32, isOutput=False)
    # [m, b, p, sc, c]: per-(m,b) output is fully contiguous; host unpermutes
    out_d = nc.declare_dram_parameter("out", [M, BL, 128, SC, C], bf16, isOutput=True)

    with tile.TileContext(nc) as tc:
        with (
            tc.tile_pool(name="xin", bufs=2) as xin_pool,
            tc.tile_pool(name="wpool", bufs=2) as w_pool,
            tc.tile_pool(name="zt", bufs=3) as zt_pool,
            tc.tile_pool(name="usb", bufs=4) as u_pool,
            tc.tile_pool(name="pz", bufs=2, space="PSUM") as pz_pool,
            tc.tile_pool(name="pu", bufs=3, space="PSUM") as pu_pool,
        ):
            for b in range(BL):
                xt_sb = xin_pool.tile([128, KC, S], bf16, tag="xt")
                if variant == 1 and b == 0:
                    pass  # issued below, after the j0 weight chunks
                else:
                    nc.gpsimd.dma_start(xt_sb[:], xt_d[b])
                wd_sb = w_pool.tile([128, JP, KC, 128], bf16, tag="wd")
                wu_sb = w_pool.tile([128, JP, C], bf16, tag="wu")
                bias_sb = w_pool.tile([128, JP], f32, tag="bias")
                # first pair's weights land first so PE can start the
                # batch's j=0 matmuls without waiting for the full tables
                # (cuts the PE stall at batch boundaries that re-throttles
                # the HAM clock gate)
                eng0 = nc.sync if (variant == 1 and b == 0) else nc.gpsimd
                eng0.dma_start(wd_sb[:, 0:1], wd_d[b, :, 0:1])
                if variant == 1 and b == 0:
                    nc.sync.dma_start(xt_sb[:], xt_d[b])
                eng0.dma_start(wu_sb[:, 0], wu_d[b, :, 0])
                eng0.dma_start(bias_sb[:], bias_d[b])
                nc.gpsimd.dma_start(wd_sb[:, 1:JP], wd_d[b, :, 1:JP])
                nc.gpsimd.dma_start(wu_sb[:, 1:JP], wu_d[b, :, 1:JP])

                for j in range(JP):
                    m0, m1 = 2 * j, 2 * j + 1
                    # down-proj, col-packed: m0 -> PE cols 0-63 -> psum
                    # partitions 0-63; m1 -> cols 64-127.
                    # both routers' down-weights are stacked along the
                    # stationary free dim on host, so one full-array matmul
                    # per k-chunk computes the pair (half the instructions,
                    # FWL-eligible 128-col weight loads)
                    psum_z = pz_pool.tile([128, S], f32, tag="pz")
                    for k in range(KC):
                        nc.tensor.matmul(
                            psum_z[:],
                            lhsT=wd_sb[:, j, k, :],
                            rhs=xt_sb[:, k, :],
                            start=(k == 0),
                            stop=(k == KC - 1),
                        )
                    # SiLU(z + bias) for both routers in one op, cast to bf16
                    zt_sb = zt_pool.tile([128, S], bf16, tag="zt")
                    nc.scalar.activation(
                        zt_sb[:], psum_z[:], AF.Silu, bias=bias_sb[:, j : j + 1]
                    )
                    # up-proj, row-packed: m0 -> PE rows 0-63, m1 -> rows
                    # 64-127, interleaved so the array works on both at once.
                    u0 = u_pool.tile([128, SC, C], bf16, tag="u0")
                    u1 = u_pool.tile([128, SC, C], bf16, tag="u1")
                    ev = j % 2
                    for sc in range(SC):
                        p0 = pu_pool.tile([128, C], f32, tag="pu")
                        p1 = pu_pool.tile([128, C], f32, tag="pu")
                        for cc in range(2):
                            nc.tensor.matmul(
                                p0[:, cc * 512 : (cc + 1) * 512],
                                lhsT=zt_sb[0:64, sc * 128 : (sc + 1) * 128],
                                rhs=wu_sb[0:64, j, cc * 512 : (cc + 1) * 512],
                                start=True,
                                stop=True,
                                tile_position=(0, 0),
                            )
                            nc.tensor.matmul(
                                p1[:, cc * 512 : (cc + 1) * 512],
                                lhsT=zt_sb[64:128, sc * 128 : (sc + 1) * 128],
                                rhs=wu_sb[64:128, j, cc * 512 : (cc + 1) * 512],
                                start=True,
                                stop=True,
                                tile_position=(64, 0),
                            )
                        tail = b == BL - 1 and j >= JP - 2
                        for pt, ut in ((p0, u0), (p1, u1)):
                            dst = ut[:, sc, :]
                            if tail:
                                # kernel tail is eviction-chain-bound: split
                                # each eviction across both engines so the
                                # last tiles drain twice as fast
                                nc.scalar.copy(dst[:, 0:512], pt[:, 0:512])
                                nc.vector.tensor_copy(dst[:, 512:C], pt[:, 512:C])
                            elif ev % 2 == 0:
                                nc.scalar.copy(dst, pt[:])
                            else:
                                nc.vector.tensor_copy(dst, pt[:])
                            ev += 1
                    if variant == 1 and b == BL - 1 and j == JP - 1:
                        for sc in range(SC):
                            nc.sync.dma_start(out_d[m0, b, :, sc], u0[:, sc])
                            nc.sync.dma_start(out_d[m1, b, :, sc], u1[:, sc])
                    else:
                        for half in range(2):
                            hs = slice(half * 2, half * 2 + 2)
                            nc.sync.dma_start(out_d[m0, b, :, hs], u0[:, hs])
                            nc.sync.dma_start(out_d[m1, b, :, hs], u1[:, hs])
    nc.compile()
    return nc


def _build2(u_bufs=5, w_bufs=3, x_bufs=3):
    """Variant 2: flat (b, j) pipeline tuned so the DMA queues never starve.

    Trace lessons baked in here:
      - ALL inputs ride one gpsimd SWDGE FIFO ordered by need-time, with
        per-BATCH weight tiles (3MB) loaded in 1MB chunks: the input queue
        carries multi-MB standing backlog that rides through compute jitter
        (the v2 per-pair loads left the queue empty between pairs and every
        compute hiccup became a DMA gap).
      - outputs on the sync HWDGE queue at 512KB granularity (issued per
        sc-pair as evictions complete), so output issuance tracks the
        eviction pipeline instead of waiting for a whole 2MB unit.
      - u pool is deep: compute has >=17% headroom over the 7.5us/unit DMA
        pace, sprints ahead, and the kernel tail is pure queued-DMA drain
        with no compute dependency (v1 lost ~6us to eviction-chained dips
        in the last batch).
      - the Silu ACT table is preloaded by a dummy activation at t~0 (it
        otherwise lazy-loads for 1.3us on the first unit's critical path).
      - out DRAM layout [BL, 128, JP, 2, SC, C]: each 512KB DMA is
        4KB-contiguous per partition, host unpermutes.
    """
    import concourse.mybir as mybir
    from concourse import bacc, tile

    bf16 = mybir.dt.bfloat16
    f32 = mybir.dt.float32
    AF = mybir.ActivationFunctionType

    nc = bacc.Bacc(
        "TRN2",
        target_bir_lowering=False,
        debug=False,
        num_devices=NCORES,
        num_swdge_queues=4,
    )
    xt_d = nc.declare_dram_parameter("xt", [BL, 128, KC, S], bf16, isOutput=False)
    w_d = nc.declare_dram_parameter("w", [BL, 128, JP, 2048], bf16, isOutput=False)
    bias_d = nc.declare_dram_parameter("bias", [128, BL * JP], f32, isOutput=False)
    out_d = nc.declare_dram_parameter(
        "out", [BL, 128, JP, 2, SC, C], bf16, isOutput=True
    )

    with tile.TileContext(nc) as tc:
        with (
            tc.tile_pool(name="xin", bufs=x_bufs) as xin_pool,
            tc.tile_pool(name="wpool", bufs=w_bufs) as w_pool,
            tc.tile_pool(name="bias", bufs=1) as bias_pool,
            tc.tile_pool(name="zt", bufs=3) as zt_pool,
            tc.tile_pool(name="usb", bufs=u_bufs) as u_pool,
            tc.tile_pool(name="pz", bufs=2, space="PSUM") as pz_pool,
            tc.tile_pool(name="pu", bufs=3, space="PSUM") as pu_pool,
        ):
            bias_sb = bias_pool.tile([128, BL * JP], f32, tag="bias")
            nc.sync.dma_start(bias_sb[:], bias_d[:])
            # dummy activation: pull in the Silu table while inputs stream
            warm_sb = bias_pool.tile([128, 1], bf16, tag="warm")
            nc.scalar.activation(warm_sb[:], bias_sb[:, 0:1], AF.Silu)

            xt_tiles = {}
            w_tiles = {}

            def load_batch(b, split_x):
                xt_nb = xin_pool.tile([128, KC, S], bf16, tag="xt")
                xt_tiles[b] = xt_nb
                w_nb = w_pool.tile([128, JP, 2048], bf16, tag="w")
                w_tiles[b] = w_nb
                if split_x:
                    # head: j0's weights + first half of x land first so the
                    # first unit starts after ~1MB, not 4MB. These ride the
                    # sync HWDGE queue because it clears the runtime preamble
                    # ~5us before gpsimd's SWDGE does (and carries no output
                    # DMAs yet).
                    nc.sync.dma_start(w_nb[:, 0:1], w_d[b, :, 0:1])
                    nc.sync.dma_start(xt_nb[:, 0:4], xt_d[b, :, 0:4])
                    nc.sync.dma_start(xt_nb[:, 4:KC], xt_d[b, :, 4:KC])
                    nc.sync.dma_start(w_nb[:, 1:3], w_d[b, :, 1:3])
                    nc.gpsimd.dma_start(w_nb[:, 3:JP], w_d[b, :, 3:JP])
                else:
                    nc.gpsimd.dma_start(xt_nb[:], xt_d[b])
                    nc.gpsimd.dma_start(w_nb[:, 0:2], w_d[b, :, 0:2])
                    nc.gpsimd.dma_start(w_nb[:, 2:4], w_d[b, :, 2:4])
                    nc.gpsimd.dma_start(w_nb[:, 4:JP], w_d[b, :, 4:JP])

            load_batch(0, split_x=True)
            for b in range(BL):
                xt_sb = xt_tiles[b]
                w_sb = w_tiles[b]
                for j in range(JP):
                    if j == 1 and b + 1 < BL:
                        load_batch(b + 1, split_x=False)
                    psum_z = pz_pool.tile([128, S], f32, tag="pz")
                    for k in range(KC):
                        nc.tensor.matmul(
                            psum_z[:],
                            lhsT=w_sb[:, j, k * 128 : (k + 1) * 128],
                            rhs=xt_sb[:, k, :],
                            start=(k == 0),
                            stop=(k == KC - 1),
                        )
                    zt_sb = zt_pool.tile([128, S], bf16, tag="zt")
                    ui = b * JP + j
                    nc.scalar.activation(
                        zt_sb[:], psum_z[:], AF.Silu,
                        bias=bias_sb[:, ui : ui + 1],
                    )
                    u_sb = u_pool.tile([128, 2, SC, C], bf16, tag="u")
                    ev = j
                    for sc in range(SC):
                        p0 = pu_pool.tile([128, C], f32, tag="pu")
                        p1 = pu_pool.tile([128, C], f32, tag="pu")
                        for cc in range(2):
                            nc.tensor.matmul(
                                p0[:, cc * 512 : (cc + 1) * 512],
                                lhsT=zt_sb[0:64, sc * 128 : (sc + 1) * 128],
                                rhs=w_sb[0:64, j, 1024 + cc * 512 : 1024 + (cc + 1) * 512],
                                start=True,
                                stop=True,
                                tile_position=(0, 0),
                            )
                            nc.tensor.matmul(
                                p1[:, cc * 512 : (cc + 1) * 512],
                                lhsT=zt_sb[64:128, sc * 128 : (sc + 1) * 128],
                                rhs=w_sb[64:128, j, 1024 + cc * 512 : 1024 + (cc + 1) * 512],
                                start=True,
                                stop=True,
                                tile_position=(64, 0),
                            )
                        for half, pt in ((0, p0), (1, p1)):
                            dst = u_sb[:, half, sc, :]
                            if ev % 2 == 0:
                                nc.scalar.copy(dst, pt[:])
                            else:
                                nc.vector.tensor_copy(dst, pt[:])
                            ev += 1
                        if sc == 1 or sc == 3:
                            hs = slice(sc - 1, sc + 1)
                            nc.sync.dma_start(
                                out_d[b, :, j, :, hs], u_sb[:, :, hs, :]
                            )
    nc.compile()
    return nc


def _get_nc(variant=0):
    if variant not in _nc_cache:
        if variant == 2:
            _nc_cache[variant] = _build2()
        else:
            _nc_cache[variant] = _build(variant)
    return _nc_cache[variant]


def kernel(x, expert_index, down_w, down_b, up_w):
    global last_results
    from concourse.bass_utils import run_bass_kernel_spmd

    x = np.asarray(x, dtype=np.float32)              # [B, S, C]
    idx = np.asarray(expert_index).astype(np.int64)  # [M, B]
    down_w = np.asarray(down_w, dtype=np.float32)    # [M, N, C, D]
    down_b = np.asarray(down_b, dtype=np.float32)    # [M, N, D]
    up_w = np.asarray(up_w, dtype=np.float32)        # [M, N, D, C]

    m_idx = np.arange(M)[:, None]
    wd_g = down_w[m_idx, idx]                        # [M, B, C, D]
    bb_g = down_b[m_idx, idx]                        # [M, B, D]
    wu_g = up_w[m_idx, idx]                          # [M, B, D, C]

    variant = int(os.environ.get("KERNEL_VARIANT", "2"))

    # xt[b, p, k, s] = x[b, s, k*128+p]
    xt = np.ascontiguousarray(
        x.transpose(0, 2, 1).reshape(B, KC, 128, S).transpose(0, 2, 1, 3)
    ).astype(BF16)
    # wd[b, p, j, k, dd]: dd in [0,128) holds router 2j (d=dd) in the low
    # 64 columns and router 2j+1 (d=dd-64) in the high 64 columns, so one
    # [128,128] stationary load covers the pair
    wd = np.ascontiguousarray(
        wd_g.reshape(JP, 2, B, KC, 128, D)
        .transpose(2, 4, 0, 3, 1, 5)
        .reshape(B, 128, JP, KC, 128)
    ).astype(BF16)
    # wu[b, p, j, c]: partitions 0-63 hold router 2j (d = p), partitions
    # 64-127 hold router 2j+1 (d = p-64)
    wu_p = wu_g.reshape(JP, 2, B, D, C).transpose(2, 1, 3, 0, 4)  # [B,2,D,JP,C]
    wu = np.ascontiguousarray(wu_p.reshape(B, 128, JP, C)).astype(BF16)
    # bias[b, p, j], same partition packing as wu
    bias_p = bb_g.reshape(JP, 2, B, D).transpose(2, 1, 3, 0)      # [B,2,D,JP]
    bias = np.ascontiguousarray(bias_p.reshape(B, 128, JP)).astype(np.float32)

    in_maps = []
    if variant == 2:
        # combined weight tile per (b, j): cols 0-1024 = down (k-major),
        # cols 1024-2048 = up
        w_all = np.ascontiguousarray(
            np.concatenate([wd.reshape(B, 128, JP, KC * 128), wu], axis=-1)
        )
        bias_full = bias.transpose(1, 0, 2)        # [128, B, JP]
        for core in range(NCORES):
            sl = slice(core * BL, (core + 1) * BL)
            in_maps.append(
                {
                    "xt": xt[sl],
                    "w": w_all[sl],
                    "bias": np.ascontiguousarray(bias_full[:, sl, :]).reshape(
                        128, BL * JP
                    ),
                }
            )
    else:
        for core in range(NCORES):
            sl = slice(core * BL, (core + 1) * BL)
            in_maps.append(
                {
                    "xt": xt[sl],
                    "wd": wd[sl],
                    "wu": wu[sl],
                    "bias": bias[sl],
                }
            )

    nc = _get_nc(variant)
    trace_kwargs = {}
    if os.environ.get("KERNEL_TRACE_ALL"):
        trace_kwargs["trace_cores"] = list(range(NCORES))
    res = None
    for attempt in range(3):
        try:
            res = run_bass_kernel_spmd(
                nc, in_maps, core_ids=list(range(NCORES)), trace=TRACE, **trace_kwargs
            )
            break
        except Exception:
            # transient NRT_EXEC_UNIT_UNRECOVERABLE has been observed on a
            # process's first execute (stale device state from a prior
            # process); give the runtime a moment to recover, then retry
            if attempt == 2:
                raise
            time.sleep(10.0 * (attempt + 1))
    last_results = res

    out = np.empty((M, B, S, C), dtype=np.float32)
    for core in range(NCORES):
        sl = slice(core * BL, (core + 1) * BL)
        dev = res.results[core]["out"]
        if variant == 2:
            # dev out [BL, p, j, h, sc, c] -> [m = 2j+h, BL, s = sc*128+p, c]
            out[:, sl] = (
                dev.transpose(2, 3, 0, 4, 1, 5)
                .reshape(M, BL, S, C)
                .astype(np.float32)
            )
        else:
            # dev out [M, BL, p, sc, c] -> [M, BL, s = sc*128+p, c]
            out[:, sl] = dev.transpose(0, 1, 3, 2, 4).reshape(M, BL, S, C).astype(
                np.float32
            )
    return out



# revision 20
# speedup vs baseline: 1.0055x; 1.0055x over previous
"""Trainium2 Bass kernel for nn_Adapter (moe_routing).

Reference computation (per router m in [0,12), batch b in [0,32)):
    e = expert_index[m, b]
    z = x[b] @ down_w[m, e] + down_b[m, e]     # [S, D]
    z = z * sigmoid(z)                          # SiLU
    u[m, b] = z @ up_w[m, e]                    # [S, C]

Strategy:
  - Data-parallel over batch B=32 across 8 cores (4 batches per core).
  - Expert routing (the gather over expert_index) is done on HOST: each
    core receives the already-gathered per-(m,b) weight tables, laid out
    exactly as the SBUF tiles want them, pre-cast to bf16.
  - Device, per (b, m-pair): routers are processed two at a time packed
    into the 128x128 PE array:
      * down-proj: z^T[D=64,S] for m0 -> PE cols 0-63, m1 -> cols 64-127
        (col tiling), accumulating over 8 K-chunks of C=1024.
      * SiLU+bias on the combined [128,S] PSUM tile in one ScalarE op,
        output bf16.
      * up-proj: m0 -> PE rows 0-63, m1 -> rows 64-127 (row tiling),
        K=D=64, interleaved so consecutive instructions overlap in the
        array.
    PSUM evicted to SBUF as bf16 by ScalarE/VectorE (alternating);
    two fully contiguous 512KiB DMAs out per (m,b) in [p, sc, c]
    layout which the host unpermutes. Input prefetch rides gpsimd's
    SWDGE queue so it never queues behind blocking output-DMA waits
    on the sync engine's HWDGE queue.
  - Output returned to host as bf16, host upcasts to f32 and stitches.
  - Perf: ~52 GFLOP as bf16 on 8 cores; per core 67.3MB of DRAM
    traffic is the roofline (~188us at the 358 GB/s per-core HBM spec);
    measured 181-224us run-to-run depending on chip-level HBM mode
    (337-383 GB/s effective), DMA busy 91-95% of the span.
"""

import os
import sys
import time

sys.path.insert(0, "/opt/trn_rl_repo")

import numpy as np
import ml_dtypes

M, N_EXP, C, D = 12, 8, 1024, 64
B, S = 32, 512
NCORES = 8
BL = B // NCORES          # batches per core = 4
KC = C // 128             # contraction chunks for down-proj = 8
SC = S // 128             # output row chunks for up-proj = 4
JP = M // 2               # router pairs per batch = 6

BF16 = ml_dtypes.bfloat16

# set by test.py to collect the profile
TRACE = bool(os.environ.get("KERNEL_TRACE"))
last_results = None

_nc_cache = {}


def _ensure_ntff_hook():
    """The agent image's `antenv` lacks `axon_hooks`, so the boot-time NTFF
    profile hook registration degrades silently and bass_utils' trace path
    crashes on import. Shim the module and install the ctypes hook."""
    import types

    if "antenv.axon_hooks" in sys.modules:
        return
    mod = types.ModuleType("antenv.axon_hooks")
    store = [None]
    mod.set_axon_ntff_profile_hook = lambda h: store.__setitem__(0, h)
    mod.get_axon_ntff_profile_hook = lambda: store[0]
    sys.modules["antenv.axon_hooks"] = mod
    try:
        import antenv

        antenv.axon_hooks = mod
    except ImportError:
        pass
    try:
        from trn_agent_boot.trn_boot import _ntff_profile_via_ctypes

        so_path = "/opt/axon/libaxon_pjrt.so"
        if os.path.exists(so_path):
            hook = _ntff_profile_via_ctypes(so_path)
            if hook is not None:
                mod.set_axon_ntff_profile_hook(hook)
    except Exception:
        pass


_ensure_ntff_hook()


def _build(variant=0):
    import concourse.mybir as mybir
    from concourse import bacc, tile

    bf16 = mybir.dt.bfloat16
    f32 = mybir.dt.float32
    AF = mybir.ActivationFunctionType

    nc = bacc.Bacc(
        "TRN2",
        target_bir_lowering=False,
        debug=False,
        num_devices=NCORES,
        num_swdge_queues=4,
    )
    xt_d = nc.declare_dram_parameter("xt", [BL, 128, KC, S], bf16, isOutput=False)
    wd_d = nc.declare_dram_parameter("wd", [BL, 128, JP, KC, 128], bf16, isOutput=False)
    wu_d = nc.declare_dram_parameter("wu", [BL, 128, JP, C], bf16, isOutput=False)
    bias_d = nc.declare_dram_parameter("bias", [BL, 128, JP], f32, isOutput=False)
    # [m, b, p, sc, c]: per-(m,b) output is fully contiguous; host unpermutes
    out_d = nc.declare_dram_parameter("out", [M, BL, 128, SC, C], bf16, isOutput=True)

    with tile.TileContext(nc) as tc:
        with (
            tc.tile_pool(name="xin", bufs=2) as xin_pool,
            tc.tile_pool(name="wpool", bufs=2) as w_pool,
            tc.tile_pool(name="zt", bufs=3) as zt_pool,
            tc.tile_pool(name="usb", bufs=4) as u_pool,
            tc.tile_pool(name="pz", bufs=2, space="PSUM") as pz_pool,
            tc.tile_pool(name="pu", bufs=3, space="PSUM") as pu_pool,
        ):
            for b in range(BL):
                xt_sb = xin_pool.tile([128, KC, S], bf16, tag="xt")
                if variant == 1 and b == 0:
                    pass  # issued below, after the j0 weight chunks
                else:
                    nc.gpsimd.dma_start(xt_sb[:], xt_d[b])
                wd_sb = w_pool.tile([128, JP, KC, 128], bf16, tag="wd")
                wu_sb = w_pool.tile([128, JP, C], bf16, tag="wu")
                bias_sb = w_pool.tile([128, JP], f32, tag="bias")
                # first pair's weights land first so PE can start the
                # batch's j=0 matmuls without waiting for the full tables
                # (cuts the PE stall at batch boundaries that re-throttles
                # the HAM clock gate)
                eng0 = nc.sync if (variant == 1 and b == 0) else nc.gpsimd
                eng0.dma_start(wd_sb[:, 0:1], wd_d[b, :, 0:1])
                if variant == 1 and b == 0:
                    nc.sync.dma_start(xt_sb[:], xt_d[b])
                eng0.dma_start(wu_sb[:, 0], wu_d[b, :, 0])
                eng0.dma_start(bias_sb[:], bias_d[b])
                nc.gpsimd.dma_start(wd_sb[:, 1:JP], wd_d[b, :, 1:JP])
                nc.gpsimd.dma_start(wu_sb[:, 1:JP], wu_d[b, :, 1:JP])

                for j in range(JP):
                    m0, m1 = 2 * j, 2 * j + 1
                    # down-proj, col-packed: m0 -> PE cols 0-63 -> psum
                    # partitions 0-63; m1 -> cols 64-127.
                    # both routers' down-weights are stacked along the
                    # stationary free dim on host, so one full-array matmul
                    # per k-chunk computes the pair (half the instructions,
                    # FWL-eligible 128-col weight loads)
                    psum_z = pz_pool.tile([128, S], f32, tag="pz")
                    for k in range(KC):
                        nc.tensor.matmul(
                            psum_z[:],
                            lhsT=wd_sb[:, j, k, :],
                            rhs=xt_sb[:, k, :],
                            start=(k == 0),
                            stop=(k == KC - 1),
                        )
                    # SiLU(z + bias) for both routers in one op, cast to bf16
                    zt_sb = zt_pool.tile([128, S], bf16, tag="zt")
                    nc.scalar.activation(
                        zt_sb[:], psum_z[:], AF.Silu, bias=bias_sb[:, j : j + 1]
                    )
                    # up-proj, row-packed: m0 -> PE rows 0-63, m1 -> rows
                    # 64-127, interleaved so the array works on both at once.
                    u0 = u_pool.tile([128, SC, C], bf16, tag="u0")
                    u1 = u_pool.tile([128, SC, C], bf16, tag="u1")
                    ev = j % 2
                    for sc in range(SC):
                        p0 = pu_pool.tile([128, C], f32, tag="pu")
                        p1 = pu_pool.tile([128, C], f32, tag="pu")
                        for cc in range(2):
                            nc.tensor.matmul(
                                p0[:, cc * 512 : (cc + 1) * 512],
                                lhsT=zt_sb[0:64, sc * 128 : (sc + 1) * 128],
                                rhs=wu_sb[0:64, j, cc * 512 : (cc + 1) * 512],
                                start=True,
                                stop=True,
                                tile_position=(0, 0),
                            )
                            nc.tensor.matmul(
                                p1[:, cc * 512 : (cc + 1) * 512],
                                lhsT=zt_sb[64:128, sc * 128 : (sc + 1) * 128],
                                rhs=wu_sb[64:128, j, cc * 512 : (cc + 1) * 512],
                                start=True,
                                stop=True,
                                tile_position=(64, 0),
                            )
                        tail = b == BL - 1 and j >= JP - 2
                        for pt, ut in ((p0, u0), (p1, u1)):
                            dst = ut[:, sc, :]
                            if tail:
                                # kernel tail is eviction-chain-bound: split
                                # each eviction across both engines so the
                                # last tiles drain twice as fast
                                nc.scalar.copy(dst[:, 0:512], pt[:, 0:512])
                                nc.vector.tensor_copy(dst[:, 512:C], pt[:, 512:C])
                            elif ev % 2 == 0:
                                nc.scalar.copy(dst, pt[:])
                            else:
                                nc.vector.tensor_copy(dst, pt[:])
                            ev += 1
                    if variant == 1 and b == BL - 1 and j == JP - 1:
                        for sc in range(SC):
                            nc.sync.dma_start(out_d[m0, b, :, sc], u0[:, sc])
                            nc.sync.dma_start(out_d[m1, b, :, sc], u1[:, sc])
                    else:
                        for half in range(2):
                            hs = slice(half * 2, half * 2 + 2)
                            nc.sync.dma_start(out_d[m0, b, :, hs], u0[:, hs])
                            nc.sync.dma_start(out_d[m1, b, :, hs], u1[:, hs])
    nc.compile()
    return nc


def _build2(u_bufs=6, w_bufs=2, x_bufs=2, sync_head=True, merged_out=True,
            prefetch_j=2):
    """Variant 2: flat (b, j) pipeline tuned so the DMA queues never starve.

    Trace lessons baked in here:
      - ALL inputs ride one gpsimd SWDGE FIFO ordered by need-time, with
        per-BATCH weight tiles (3MB) loaded in 1MB chunks: the input queue
        carries multi-MB standing backlog that rides through compute jitter
        (the v2 per-pair loads left the queue empty between pairs and every
        compute hiccup became a DMA gap).
      - outputs on the sync HWDGE queue at 512KB granularity (issued per
        sc-pair as evictions complete), so output issuance tracks the
        eviction pipeline instead of waiting for a whole 2MB unit.
      - u pool is deep: compute has >=17% headroom over the 7.5us/unit DMA
        pace, sprints ahead, and the kernel tail is pure queued-DMA drain
        with no compute dependency (v1 lost ~6us to eviction-chained dips
        in the last batch).
      - the Silu ACT table is preloaded by a dummy activation at t~0 (it
        otherwise lazy-loads for 1.3us on the first unit's critical path).
      - out DRAM layout [BL, 128, JP, 2, SC, C]: each 512KB DMA is
        4KB-contiguous per partition, host unpermutes.
    """
    import concourse.mybir as mybir
    from concourse import bacc, tile

    bf16 = mybir.dt.bfloat16
    f32 = mybir.dt.float32
    AF = mybir.ActivationFunctionType

    nc = bacc.Bacc(
        "TRN2",
        target_bir_lowering=False,
        debug=False,
        num_devices=NCORES,
        num_swdge_queues=4,
    )
    xt_d = nc.declare_dram_parameter("xt", [BL, 128, KC, S], bf16, isOutput=False)
    w_d = nc.declare_dram_parameter("w", [BL, 128, JP, 2048], bf16, isOutput=False)
    bias_d = nc.declare_dram_parameter("bias", [128, BL * JP], f32, isOutput=False)
    out_d = nc.declare_dram_parameter(
        "out", [BL, 128, JP, 2, SC, C], bf16, isOutput=True
    )

    with tile.TileContext(nc) as tc:
        with (
            tc.tile_pool(name="xin", bufs=x_bufs) as xin_pool,
            tc.tile_pool(name="wpool", bufs=w_bufs) as w_pool,
            tc.tile_pool(name="bias", bufs=1) as bias_pool,
            tc.tile_pool(name="zt", bufs=3) as zt_pool,
            tc.tile_pool(name="usb", bufs=u_bufs) as u_pool,
            tc.tile_pool(name="pz", bufs=2, space="PSUM") as pz_pool,
            tc.tile_pool(name="pu", bufs=3, space="PSUM") as pu_pool,
        ):
            bias_sb = bias_pool.tile([128, BL * JP], f32, tag="bias")
            nc.sync.dma_start(bias_sb[:], bias_d[:])
            # dummy activation: pull in the Silu table while inputs stream
            warm_sb = bias_pool.tile([128, 1], bf16, tag="warm")
            nc.scalar.activation(warm_sb[:], bias_sb[:, 0:1], AF.Silu)

            xt_tiles = {}
            w_tiles = {}

            def load_batch(b, split_x):
                xt_nb = xin_pool.tile([128, KC, S], bf16, tag="xt")
                xt_tiles[b] = xt_nb
                w_nb = w_pool.tile([128, JP, 2048], bf16, tag="w")
                w_tiles[b] = w_nb
                if split_x:
                    # head: j0's weights + first half of x land first so the
                    # first unit starts after ~1MB, not 4MB. On sync_head
                    # these ride the HWDGE queue, which clears the runtime
                    # preamble ~5us before gpsimd's SWDGE does (and carries
                    # no output DMAs yet).
                    eng = nc.sync if sync_head else nc.gpsimd
                    eng.dma_start(w_nb[:, 0:1], w_d[b, :, 0:1])
                    eng.dma_start(xt_nb[:, 0:4], xt_d[b, :, 0:4])
                    eng.dma_start(xt_nb[:, 4:KC], xt_d[b, :, 4:KC])
                    eng.dma_start(w_nb[:, 1:3], w_d[b, :, 1:3])
                    nc.gpsimd.dma_start(w_nb[:, 3:JP], w_d[b, :, 3:JP])
                else:
                    nc.gpsimd.dma_start(xt_nb[:], xt_d[b])
                    nc.gpsimd.dma_start(w_nb[:, 0:2], w_d[b, :, 0:2])
                    nc.gpsimd.dma_start(w_nb[:, 2:4], w_d[b, :, 2:4])
                    nc.gpsimd.dma_start(w_nb[:, 4:JP], w_d[b, :, 4:JP])

            load_batch(0, split_x=True)
            for b in range(BL):
                xt_sb = xt_tiles[b]
                w_sb = w_tiles[b]
                for j in range(JP):
                    if j == prefetch_j and b + 1 < BL:
                        load_batch(b + 1, split_x=False)
                    psum_z = pz_pool.tile([128, S], f32, tag="pz")
                    for k in range(KC):
                        nc.tensor.matmul(
                            psum_z[:],
                            lhsT=w_sb[:, j, k * 128 : (k + 1) * 128],
                            rhs=xt_sb[:, k, :],
                            start=(k == 0),
                            stop=(k == KC - 1),
                        )
                    zt_sb = zt_pool.tile([128, S], bf16, tag="zt")
                    ui = b * JP + j
                    nc.scalar.activation(
                        zt_sb[:], psum_z[:], AF.Silu,
                        bias=bias_sb[:, ui : ui + 1],
                    )
                    u_sb = u_pool.tile([128, 2, SC, C], bf16, tag="u")
                    ev = j
                    for sc in range(SC):
                        p0 = pu_pool.tile([128, C], f32, tag="pu")
                        p1 = pu_pool.tile([128, C], f32, tag="pu")
                        for cc in range(2):
                            nc.tensor.matmul(
                                p0[:, cc * 512 : (cc + 1) * 512],
                                lhsT=zt_sb[0:64, sc * 128 : (sc + 1) * 128],
                                rhs=w_sb[0:64, j, 1024 + cc * 512 : 1024 + (cc + 1) * 512],
                                start=True,
                                stop=True,
                                tile_position=(0, 0),
                            )
                            nc.tensor.matmul(
                                p1[:, cc * 512 : (cc + 1) * 512],
                                lhsT=zt_sb[64:128, sc * 128 : (sc + 1) * 128],
                                rhs=w_sb[64:128, j, 1024 + cc * 512 : 1024 + (cc + 1) * 512],
                                start=True,
                                stop=True,
                                tile_position=(64, 0),
                            )
                        for half, pt in ((0, p0), (1, p1)):
                            dst = u_sb[:, half, sc, :]
                            if ev % 2 == 0:
                                nc.scalar.copy(dst, pt[:])
                            else:
                                nc.vector.tensor_copy(dst, pt[:])
                            ev += 1
                        if sc == 1 or sc == 3:
                            hs = slice(sc - 1, sc + 1)
                            if merged_out:
                                nc.sync.dma_start(
                                    out_d[b, :, j, :, hs], u_sb[:, :, hs, :]
                                )
                            else:
                                nc.sync.dma_start(
                                    out_d[b, :, j, 0, hs], u_sb[:, 0, hs]
                                )
                                nc.sync.dma_start(
                                    out_d[b, :, j, 1, hs], u_sb[:, 1, hs]
                                )
    nc.compile()
    return nc


def _get_nc(variant=0, **kwargs):
    key = (variant, tuple(sorted(kwargs.items())))
    if key not in _nc_cache:
        if variant == 2:
            _nc_cache[key] = _build2(**kwargs)
        else:
            _nc_cache[key] = _build(variant)
    return _nc_cache[key]


def _pack_inputs(x, expert_index, down_w, down_b, up_w, variant):
    x = np.asarray(x, dtype=np.float32)              # [B, S, C]
    idx = np.asarray(expert_index).astype(np.int64)  # [M, B]
    down_w = np.asarray(down_w, dtype=np.float32)    # [M, N, C, D]
    down_b = np.asarray(down_b, dtype=np.float32)    # [M, N, D]
    up_w = np.asarray(up_w, dtype=np.float32)        # [M, N, D, C]

    m_idx = np.arange(M)[:, None]
    wd_g = down_w[m_idx, idx]                        # [M, B, C, D]
    bb_g = down_b[m_idx, idx]                        # [M, B, D]
    wu_g = up_w[m_idx, idx]                          # [M, B, D, C]

    # xt[b, p, k, s] = x[b, s, k*128+p]
    xt = np.ascontiguousarray(
        x.transpose(0, 2, 1).reshape(B, KC, 128, S).transpose(0, 2, 1, 3)
    ).astype(BF16)
    # wd[b, p, j, k, dd]: dd in [0,128) holds router 2j (d=dd) in the low
    # 64 columns and router 2j+1 (d=dd-64) in the high 64 columns, so one
    # [128,128] stationary load covers the pair
    wd = np.ascontiguousarray(
        wd_g.reshape(JP, 2, B, KC, 128, D)
        .transpose(2, 4, 0, 3, 1, 5)
        .reshape(B, 128, JP, KC, 128)
    ).astype(BF16)
    # wu[b, p, j, c]: partitions 0-63 hold router 2j (d = p), partitions
    # 64-127 hold router 2j+1 (d = p-64)
    wu_p = wu_g.reshape(JP, 2, B, D, C).transpose(2, 1, 3, 0, 4)  # [B,2,D,JP,C]
    wu = np.ascontiguousarray(wu_p.reshape(B, 128, JP, C)).astype(BF16)
    # bias[b, p, j], same partition packing as wu
    bias_p = bb_g.reshape(JP, 2, B, D).transpose(2, 1, 3, 0)      # [B,2,D,JP]
    bias = np.ascontiguousarray(bias_p.reshape(B, 128, JP)).astype(np.float32)

    in_maps = []
    if variant == 2:
        # combined weight tile per (b, j): cols 0-1024 = down (k-major),
        # cols 1024-2048 = up
        w_all = np.ascontiguousarray(
            np.concatenate([wd.reshape(B, 128, JP, KC * 128), wu], axis=-1)
        )
        bias_full = bias.transpose(1, 0, 2)        # [128, B, JP]
        for core in range(NCORES):
            sl = slice(core * BL, (core + 1) * BL)
            in_maps.append(
                {
                    "xt": xt[sl],
                    "w": w_all[sl],
                    "bias": np.ascontiguousarray(bias_full[:, sl, :]).reshape(
                        128, BL * JP
                    ),
                }
            )
    else:
        for core in range(NCORES):
            sl = slice(core * BL, (core + 1) * BL)
            in_maps.append(
                {
                    "xt": xt[sl],
                    "wd": wd[sl],
                    "wu": wu[sl],
                    "bias": bias[sl],
                }
            )
    return in_maps


def kernel(x, expert_index, down_w, down_b, up_w):
    global last_results
    from concourse.bass_utils import run_bass_kernel_spmd

    variant = int(os.environ.get("KERNEL_VARIANT", "2"))
    in_maps = _pack_inputs(x, expert_index, down_w, down_b, up_w, variant)

    nc = _get_nc(variant)
    trace_kwargs = {}
    if os.environ.get("KERNEL_TRACE_ALL"):
        trace_kwargs["trace_cores"] = list(range(NCORES))
    res = None
    for attempt in range(3):
        try:
            res = run_bass_kernel_spmd(
                nc, in_maps, core_ids=list(range(NCORES)), trace=TRACE, **trace_kwargs
            )
            break
        except Exception:
            # transient NRT_EXEC_UNIT_UNRECOVERABLE has been observed on a
            # process's first execute (stale device state from a prior
            # process); give the runtime a moment to recover, then retry
            if attempt == 2:
                raise
            time.sleep(10.0 * (attempt + 1))
    last_results = res

    out = np.empty((M, B, S, C), dtype=np.float32)
    for core in range(NCORES):
        sl = slice(core * BL, (core + 1) * BL)
        dev = res.results[core]["out"]
        if variant == 2:
            # dev out [BL, p, j, h, sc, c] -> [m = 2j+h, BL, s = sc*128+p, c]
            out[:, sl] = (
                dev.transpose(2, 3, 0, 4, 1, 5)
                .reshape(M, BL, S, C)
                .astype(np.float32)
            )
        else:
            # dev out [M, BL, p, sc, c] -> [M, BL, s = sc*128+p, c]
            out[:, sl] = dev.transpose(0, 1, 3, 2, 4).reshape(M, BL, S, C).astype(
                np.float32
            )
    return out



# revision 22
# speedup vs baseline: 1.1830x; 1.1765x over previous
"""Trainium2 Bass kernel for nn_Adapter (moe_routing).

Reference computation (per router m in [0,12), batch b in [0,32)):
    e = expert_index[m, b]
    z = x[b] @ down_w[m, e] + down_b[m, e]     # [S, D]
    z = z * sigmoid(z)                          # SiLU
    u[m, b] = z @ up_w[m, e]                    # [S, C]

Strategy (variant 2, the default — see _build2):
  - Data-parallel over batch B=32 across 8 cores (4 batches per core).
  - Expert routing (the gather over expert_index) is done on HOST: each
    core receives the already-gathered per-(m,b) weight tables, laid out
    exactly as the SBUF tiles want them, pre-cast to bf16. Down+up
    weights for a router pair are fused into one [128, 2048] tile.
  - Device: flat stream of 24 (b, j=router-pair) units. Per unit:
      * down-proj: pair packed into the 128x128 PE array (m0 -> cols
        0-63, m1 -> cols 64-127), accumulating over 8 K-chunks of C.
      * SiLU+bias on the combined [128,S] PSUM tile in one ScalarE op.
      * up-proj: m0 -> PE rows 0-63, m1 -> rows 64-127 (quadrant
        tiling), PSUM evicted to SBUF bf16 by ScalarE/VectorE
        (alternating), 1MB output DMA per sc-pair as evictions land.
  - DMA discipline (this is what sets the time; the kernel is HBM-bound
    at ~66.4MB/core vs ~52 GFLOP bf16 which all engines cover with
    >=17% headroom): inputs ride one gpsimd SWDGE FIFO in need-order
    with 3-batch-deep pools so the input queue holds multi-MB backlog;
    outputs ride the sync HWDGE queue; compute sprints ahead of DMA
    into a 6-deep (12MB) output-tile pool so the kernel tail is pure
    queued-DMA drain with no compute dependency. The Silu ACT table is
    preloaded at t~0. Head/tail overhead is ~10us of fixed NEFF
    preamble/epilogue barriers.
  - Output returned to host as bf16 in [BL, p, j, h, sc, c] layout
    (4KB-contiguous per partition per DMA); host upcasts + unpermutes.
  - Perf: 66.35MB/core of DRAM traffic is the roofline. Measured
    178-222us run-to-run: the effective per-core HBM rate swings
    ~330-400 GB/s (shared/brokered device; not controllable from the
    kernel). DMA-engine union busy is ~100% of the span outside the
    ~10us fixed preamble/epilogue in both modes.
"""

import os
import sys
import time

sys.path.insert(0, "/opt/trn_rl_repo")

import numpy as np
import ml_dtypes

M, N_EXP, C, D = 12, 8, 1024, 64
B, S = 32, 512
NCORES = 8
BL = B // NCORES          # batches per core = 4
KC = C // 128             # contraction chunks for down-proj = 8
SC = S // 128             # output row chunks for up-proj = 4
JP = M // 2               # router pairs per batch = 6

BF16 = ml_dtypes.bfloat16

# set by test.py to collect the profile
TRACE = bool(os.environ.get("KERNEL_TRACE"))
last_results = None

_nc_cache = {}


def _ensure_ntff_hook():
    """The agent image's `antenv` lacks `axon_hooks`, so the boot-time NTFF
    profile hook registration degrades silently and bass_utils' trace path
    crashes on import. Shim the module and install the ctypes hook."""
    import types

    if "antenv.axon_hooks" in sys.modules:
        return
    mod = types.ModuleType("antenv.axon_hooks")
    store = [None]
    mod.set_axon_ntff_profile_hook = lambda h: store.__setitem__(0, h)
    mod.get_axon_ntff_profile_hook = lambda: store[0]
    sys.modules["antenv.axon_hooks"] = mod
    try:
        import antenv

        antenv.axon_hooks = mod
    except ImportError:
        pass
    try:
        from trn_agent_boot.trn_boot import _ntff_profile_via_ctypes

        so_path = "/opt/axon/libaxon_pjrt.so"
        if os.path.exists(so_path):
            hook = _ntff_profile_via_ctypes(so_path)
            if hook is not None:
                mod.set_axon_ntff_profile_hook(hook)
    except Exception:
        pass


_ensure_ntff_hook()


def _build(variant=0):
    import concourse.mybir as mybir
    from concourse import bacc, tile

    bf16 = mybir.dt.bfloat16
    f32 = mybir.dt.float32
    AF = mybir.ActivationFunctionType

    nc = bacc.Bacc(
        "TRN2",
        target_bir_lowering=False,
        debug=False,
        num_devices=NCORES,
        num_swdge_queues=4,
    )
    xt_d = nc.declare_dram_parameter("xt", [BL, 128, KC, S], bf16, isOutput=False)
    wd_d = nc.declare_dram_parameter("wd", [BL, 128, JP, KC, 128], bf16, isOutput=False)
    wu_d = nc.declare_dram_parameter("wu", [BL, 128, JP, C], bf16, isOutput=False)
    bias_d = nc.declare_dram_parameter("bias", [BL, 128, JP], f32, isOutput=False)
    # [m, b, p, sc, c]: per-(m,b) output is fully contiguous; host unpermutes
    out_d = nc.declare_dram_parameter("out", [M, BL, 128, SC, C], bf16, isOutput=True)

    with tile.TileContext(nc) as tc:
        with (
            tc.tile_pool(name="xin", bufs=2) as xin_pool,
            tc.tile_pool(name="wpool", bufs=2) as w_pool,
            tc.tile_pool(name="zt", bufs=3) as zt_pool,
            tc.tile_pool(name="usb", bufs=4) as u_pool,
            tc.tile_pool(name="pz", bufs=2, space="PSUM") as pz_pool,
            tc.tile_pool(name="pu", bufs=3, space="PSUM") as pu_pool,
        ):
            for b in range(BL):
                xt_sb = xin_pool.tile([128, KC, S], bf16, tag="xt")
                if variant == 1 and b == 0:
                    pass  # issued below, after the j0 weight chunks
                else:
                    nc.gpsimd.dma_start(xt_sb[:], xt_d[b])
                wd_sb = w_pool.tile([128, JP, KC, 128], bf16, tag="wd")
                wu_sb = w_pool.tile([128, JP, C], bf16, tag="wu")
                bias_sb = w_pool.tile([128, JP], f32, tag="bias")
                # first pair's weights land first so PE can start the
                # batch's j=0 matmuls without waiting for the full tables
                # (cuts the PE stall at batch boundaries that re-throttles
                # the HAM clock gate)
                eng0 = nc.sync if (variant == 1 and b == 0) else nc.gpsimd
                eng0.dma_start(wd_sb[:, 0:1], wd_d[b, :, 0:1])
                if variant == 1 and b == 0:
                    nc.sync.dma_start(xt_sb[:], xt_d[b])
                eng0.dma_start(wu_sb[:, 0], wu_d[b, :, 0])
                eng0.dma_start(bias_sb[:], bias_d[b])
                nc.gpsimd.dma_start(wd_sb[:, 1:JP], wd_d[b, :, 1:JP])
                nc.gpsimd.dma_start(wu_sb[:, 1:JP], wu_d[b, :, 1:JP])

                for j in range(JP):
                    m0, m1 = 2 * j, 2 * j + 1
                    # down-proj, col-packed: m0 -> PE cols 0-63 -> psum
                    # partitions 0-63; m1 -> cols 64-127.
                    # both routers' down-weights are stacked along the
                    # stationary free dim on host, so one full-array matmul
                    # per k-chunk computes the pair (half the instructions,
                    # FWL-eligible 128-col weight loads)
                    psum_z = pz_pool.tile([128, S], f32, tag="pz")
                    for k in range(KC):
                        nc.tensor.matmul(
                            psum_z[:],
                            lhsT=wd_sb[:, j, k, :],
                            rhs=xt_sb[:, k, :],
                            start=(k == 0),
                            stop=(k == KC - 1),
                        )
                    # SiLU(z + bias) for both routers in one op, cast to bf16
                    zt_sb = zt_pool.tile([128, S], bf16, tag="zt")
                    nc.scalar.activation(
                        zt_sb[:], psum_z[:], AF.Silu, bias=bias_sb[:, j : j + 1]
                    )
                    # up-proj, row-packed: m0 -> PE rows 0-63, m1 -> rows
                    # 64-127, interleaved so the array works on both at once.
                    u0 = u_pool.tile([128, SC, C], bf16, tag="u0")
                    u1 = u_pool.tile([128, SC, C], bf16, tag="u1")
                    ev = j % 2
                    for sc in range(SC):
                        p0 = pu_pool.tile([128, C], f32, tag="pu")
                        p1 = pu_pool.tile([128, C], f32, tag="pu")
                        for cc in range(2):
                            nc.tensor.matmul(
                                p0[:, cc * 512 : (cc + 1) * 512],
                                lhsT=zt_sb[0:64, sc * 128 : (sc + 1) * 128],
                                rhs=wu_sb[0:64, j, cc * 512 : (cc + 1) * 512],
                                start=True,
                                stop=True,
                                tile_position=(0, 0),
                            )
                            nc.tensor.matmul(
                                p1[:, cc * 512 : (cc + 1) * 512],
                                lhsT=zt_sb[64:128, sc * 128 : (sc + 1) * 128],
                                rhs=wu_sb[64:128, j, cc * 512 : (cc + 1) * 512],
                                start=True,
                                stop=True,
                                tile_position=(64, 0),
                            )
                        tail = b == BL - 1 and j >= JP - 2
                        for pt, ut in ((p0, u0), (p1, u1)):
                            dst = ut[:, sc, :]
                            if tail:
                                # kernel tail is eviction-chain-bound: split
                                # each eviction across both engines so the
                                # last tiles drain twice as fast
                                nc.scalar.copy(dst[:, 0:512], pt[:, 0:512])
                                nc.vector.tensor_copy(dst[:, 512:C], pt[:, 512:C])
                            elif ev % 2 == 0:
                                nc.scalar.copy(dst, pt[:])
                            else:
                                nc.vector.tensor_copy(dst, pt[:])
                            ev += 1
                    if variant == 1 and b == BL - 1 and j == JP - 1:
                        for sc in range(SC):
                            nc.sync.dma_start(out_d[m0, b, :, sc], u0[:, sc])
                            nc.sync.dma_start(out_d[m1, b, :, sc], u1[:, sc])
                    else:
                        for half in range(2):
                            hs = slice(half * 2, half * 2 + 2)
                            nc.sync.dma_start(out_d[m0, b, :, hs], u0[:, hs])
                            nc.sync.dma_start(out_d[m1, b, :, hs], u1[:, hs])
    nc.compile()
    return nc


def _build2(u_bufs=6, w_bufs=3, x_bufs=3, sync_head=True, merged_out=True,
            prefetch_j=1):
    """Variant 2: flat (b, j) pipeline tuned so the DMA queues never starve.

    Trace lessons baked in here:
      - ALL inputs ride one gpsimd SWDGE FIFO ordered by need-time, with
        per-BATCH weight tiles (3MB) loaded in 1MB chunks: the input queue
        carries multi-MB standing backlog that rides through compute jitter
        (the v2 per-pair loads left the queue empty between pairs and every
        compute hiccup became a DMA gap).
      - outputs on the sync HWDGE queue at 512KB granularity (issued per
        sc-pair as evictions complete), so output issuance tracks the
        eviction pipeline instead of waiting for a whole 2MB unit.
      - u pool is deep: compute has >=17% headroom over the 7.5us/unit DMA
        pace, sprints ahead, and the kernel tail is pure queued-DMA drain
        with no compute dependency (v1 lost ~6us to eviction-chained dips
        in the last batch).
      - the Silu ACT table is preloaded by a dummy activation at t~0 (it
        otherwise lazy-loads for 1.3us on the first unit's critical path).
      - out DRAM layout [BL, 128, JP, 2, SC, C]: each 512KB DMA is
        4KB-contiguous per partition, host unpermutes.
    """
    import concourse.mybir as mybir
    from concourse import bacc, tile

    bf16 = mybir.dt.bfloat16
    f32 = mybir.dt.float32
    AF = mybir.ActivationFunctionType

    nc = bacc.Bacc(
        "TRN2",
        target_bir_lowering=False,
        debug=False,
        num_devices=NCORES,
        num_swdge_queues=4,
    )
    xt_d = nc.declare_dram_parameter("xt", [BL, 128, KC, S], bf16, isOutput=False)
    w_d = nc.declare_dram_parameter("w", [BL, 128, JP, 2048], bf16, isOutput=False)
    bias_d = nc.declare_dram_parameter("bias", [128, BL * JP], f32, isOutput=False)
    out_d = nc.declare_dram_parameter(
        "out", [BL, 128, JP, 2, SC, C], bf16, isOutput=True
    )

    with tile.TileContext(nc) as tc:
        with (
            tc.tile_pool(name="xin", bufs=x_bufs) as xin_pool,
            tc.tile_pool(name="wpool", bufs=w_bufs) as w_pool,
            tc.tile_pool(name="bias", bufs=1) as bias_pool,
            tc.tile_pool(name="zt", bufs=3) as zt_pool,
            tc.tile_pool(name="usb", bufs=u_bufs) as u_pool,
            tc.tile_pool(name="pz", bufs=2, space="PSUM") as pz_pool,
            tc.tile_pool(name="pu", bufs=3, space="PSUM") as pu_pool,
        ):
            bias_sb = bias_pool.tile([128, BL * JP], f32, tag="bias")
            nc.sync.dma_start(bias_sb[:], bias_d[:])
            # dummy activation: pull in the Silu table while inputs stream
            warm_sb = bias_pool.tile([128, 1], bf16, tag="warm")
            nc.scalar.activation(warm_sb[:], bias_sb[:, 0:1], AF.Silu)

            xt_tiles = {}
            w_tiles = {}

            def load_batch(b, split_x):
                xt_nb = xin_pool.tile([128, KC, S], bf16, tag="xt")
                xt_tiles[b] = xt_nb
                w_nb = w_pool.tile([128, JP, 2048], bf16, tag="w")
                w_tiles[b] = w_nb
                if split_x:
                    # head: j0's weights + first half of x land first so the
                    # first unit starts after ~1MB, not 4MB. On sync_head
                    # these ride the HWDGE queue, which clears the runtime
                    # preamble ~5us before gpsimd's SWDGE does (and carries
                    # no output DMAs yet).
                    eng = nc.sync if sync_head else nc.gpsimd
                    eng.dma_start(w_nb[:, 0:1], w_d[b, :, 0:1])
                    eng.dma_start(xt_nb[:, 0:4], xt_d[b, :, 0:4])
                    eng.dma_start(xt_nb[:, 4:KC], xt_d[b, :, 4:KC])
                    eng.dma_start(w_nb[:, 1:3], w_d[b, :, 1:3])
                    nc.gpsimd.dma_start(w_nb[:, 3:JP], w_d[b, :, 3:JP])
                else:
                    nc.gpsimd.dma_start(xt_nb[:], xt_d[b])
                    nc.gpsimd.dma_start(w_nb[:, 0:2], w_d[b, :, 0:2])
                    nc.gpsimd.dma_start(w_nb[:, 2:4], w_d[b, :, 2:4])
                    nc.gpsimd.dma_start(w_nb[:, 4:JP], w_d[b, :, 4:JP])

            load_batch(0, split_x=True)
            for b in range(BL):
                xt_sb = xt_tiles[b]
                w_sb = w_tiles[b]
                for j in range(JP):
                    if j == prefetch_j and b + 1 < BL:
                        load_batch(b + 1, split_x=False)
                    psum_z = pz_pool.tile([128, S], f32, tag="pz")
                    for k in range(KC):
                        nc.tensor.matmul(
                            psum_z[:],
                            lhsT=w_sb[:, j, k * 128 : (k + 1) * 128],
                            rhs=xt_sb[:, k, :],
                            start=(k == 0),
                            stop=(k == KC - 1),
                        )
                    zt_sb = zt_pool.tile([128, S], bf16, tag="zt")
                    ui = b * JP + j
                    nc.scalar.activation(
                        zt_sb[:], psum_z[:], AF.Silu,
                        bias=bias_sb[:, ui : ui + 1],
                    )
                    u_sb = u_pool.tile([128, 2, SC, C], bf16, tag="u")
                    ev = j
                    for sc in range(SC):
                        p0 = pu_pool.tile([128, C], f32, tag="pu")
                        p1 = pu_pool.tile([128, C], f32, tag="pu")
                        for cc in range(2):
                            nc.tensor.matmul(
                                p0[:, cc * 512 : (cc + 1) * 512],
                                lhsT=zt_sb[0:64, sc * 128 : (sc + 1) * 128],
                                rhs=w_sb[0:64, j, 1024 + cc * 512 : 1024 + (cc + 1) * 512],
                                start=True,
                                stop=True,
                                tile_position=(0, 0),
                            )
                            nc.tensor.matmul(
                                p1[:, cc * 512 : (cc + 1) * 512],
                                lhsT=zt_sb[64:128, sc * 128 : (sc + 1) * 128],
                                rhs=w_sb[64:128, j, 1024 + cc * 512 : 1024 + (cc + 1) * 512],
                                start=True,
                                stop=True,
                                tile_position=(64, 0),
                            )
                        for half, pt in ((0, p0), (1, p1)):
                            dst = u_sb[:, half, sc, :]
                            if ev % 2 == 0:
                                nc.scalar.copy(dst, pt[:])
                            else:
                                nc.vector.tensor_copy(dst, pt[:])
                            ev += 1
                        if sc == 1 or sc == 3:
                            hs = slice(sc - 1, sc + 1)
                            if merged_out:
                                nc.sync.dma_start(
                                    out_d[b, :, j, :, hs], u_sb[:, :, hs, :]
                                )
                            else:
                                nc.sync.dma_start(
                                    out_d[b, :, j, 0, hs], u_sb[:, 0, hs]
                                )
                                nc.sync.dma_start(
                                    out_d[b, :, j, 1, hs], u_sb[:, 1, hs]
                                )
    nc.compile()
    return nc


def _get_nc(variant=0, **kwargs):
    key = (variant, tuple(sorted(kwargs.items())))
    if key not in _nc_cache:
        if variant == 2:
            _nc_cache[key] = _build2(**kwargs)
        else:
            _nc_cache[key] = _build(variant)
    return _nc_cache[key]


def _pack_inputs(x, expert_index, down_w, down_b, up_w, variant):
    x = np.asarray(x, dtype=np.float32)              # [B, S, C]
    idx = np.asarray(expert_index).astype(np.int64)  # [M, B]
    down_w = np.asarray(down_w, dtype=np.float32)    # [M, N, C, D]
    down_b = np.asarray(down_b, dtype=np.float32)    # [M, N, D]
    up_w = np.asarray(up_w, dtype=np.float32)        # [M, N, D, C]

    m_idx = np.arange(M)[:, None]
    wd_g = down_w[m_idx, idx]                        # [M, B, C, D]
    bb_g = down_b[m_idx, idx]                        # [M, B, D]
    wu_g = up_w[m_idx, idx]                          # [M, B, D, C]

    # xt[b, p, k, s] = x[b, s, k*128+p]
    xt = np.ascontiguousarray(
        x.transpose(0, 2, 1).reshape(B, KC, 128, S).transpose(0, 2, 1, 3)
    ).astype(BF16)
    # wd[b, p, j, k, dd]: dd in [0,128) holds router 2j (d=dd) in the low
    # 64 columns and router 2j+1 (d=dd-64) in the high 64 columns, so one
    # [128,128] stationary load covers the pair
    wd = np.ascontiguousarray(
        wd_g.reshape(JP, 2, B, KC, 128, D)
        .transpose(2, 4, 0, 3, 1, 5)
        .reshape(B, 128, JP, KC, 128)
    ).astype(BF16)
    # wu[b, p, j, c]: partitions 0-63 hold router 2j (d = p), partitions
    # 64-127 hold router 2j+1 (d = p-64)
    wu_p = wu_g.reshape(JP, 2, B, D, C).transpose(2, 1, 3, 0, 4)  # [B,2,D,JP,C]
    wu = np.ascontiguousarray(wu_p.reshape(B, 128, JP, C)).astype(BF16)
    # bias[b, p, j], same partition packing as wu
    bias_p = bb_g.reshape(JP, 2, B, D).transpose(2, 1, 3, 0)      # [B,2,D,JP]
    bias = np.ascontiguousarray(bias_p.reshape(B, 128, JP)).astype(np.float32)

    in_maps = []
    if variant == 2:
        # combined weight tile per (b, j): cols 0-1024 = down (k-major),
        # cols 1024-2048 = up
        w_all = np.ascontiguousarray(
            np.concatenate([wd.reshape(B, 128, JP, KC * 128), wu], axis=-1)
        )
        bias_full = bias.transpose(1, 0, 2)        # [128, B, JP]
        for core in range(NCORES):
            sl = slice(core * BL, (core + 1) * BL)
            in_maps.append(
                {
                    "xt": xt[sl],
                    "w": w_all[sl],
                    "bias": np.ascontiguousarray(bias_full[:, sl, :]).reshape(
                        128, BL * JP
                    ),
                }
            )
    else:
        for core in range(NCORES):
            sl = slice(core * BL, (core + 1) * BL)
            in_maps.append(
                {
                    "xt": xt[sl],
                    "wd": wd[sl],
                    "wu": wu[sl],
                    "bias": bias[sl],
                }
            )
    return in_maps


def kernel(x, expert_index, down_w, down_b, up_w):
    global last_results
    from concourse.bass_utils import run_bass_kernel_spmd

    variant = int(os.environ.get("KERNEL_VARIANT", "2"))
    in_maps = _pack_inputs(x, expert_index, down_w, down_b, up_w, variant)

    nc = _get_nc(variant)
    trace_kwargs = {}
    if os.environ.get("KERNEL_TRACE_ALL"):
        trace_kwargs["trace_cores"] = list(range(NCORES))
    res = None
    for attempt in range(3):
        try:
            res = run_bass_kernel_spmd(
                nc, in_maps, core_ids=list(range(NCORES)), trace=TRACE, **trace_kwargs
            )
            break
        except Exception:
            # transient NRT_EXEC_UNIT_UNRECOVERABLE has been observed on a
            # process's first execute (stale device state from a prior
            # process); give the runtime a moment to recover, then retry
            if attempt == 2:
                raise
            time.sleep(10.0 * (attempt + 1))
    last_results = res

    out = np.empty((M, B, S, C), dtype=np.float32)
    for core in range(NCORES):
        sl = slice(core * BL, (core + 1) * BL)
        dev = res.results[core]["out"]
        if variant == 2:
            # dev out [BL, p, j, h, sc, c] -> [m = 2j+h, BL, s = sc*128+p, c]
            out[:, sl] = (
                dev.transpose(2, 3, 0, 4, 1, 5)
                .reshape(M, BL, S, C)
                .astype(np.float32)
            )
        else:
            # dev out [M, BL, p, sc, c] -> [M, BL, s = sc*128+p, c]
            out[:, sl] = dev.transpose(0, 1, 3, 2, 4).reshape(M, BL, S, C).astype(
                np.float32
            )
    return out

